# revision 6
# baseline (speedup 1.0000x reference)
"""Trainium2 Bass kernel for the blended-attention module.

Math (per batch b, head h):
  q,k,v = per-head projections of x           [N, 64]
  S = q @ k.T * scale                         [N, N]
  cur = softmax(S, axis=-1)
  bias = abs(relu(concat(q,k) @ w1.T + b1) @ w2.T + b2)   [N, 3]  (per-row)
  attn = cur*(1+bias0) + prev*bias1 - bias2/N
  out_head = attn @ v ;  out = concat_heads @ proj_w.T + proj_b
Returns (out, cur).

Key restructuring: bias0/1/2 are per-row scalars, so
  out_head = diag((1+b0)/s) @ (P@v) + diag(b1) @ (prev@v) - diag(b2/N) @ (1.T@v)
with P = exp(S*scale) unnormalized, s = rowsum(P).  No blended-attn
materialization; prev_attn is consumed only by a matmul.

Everything on-device is computed in the transposed orientation (S^T [m, n],
P^T, O^T) so the PE contraction axes line up without any on-device
transposes; the host feeds prev_attn pre-transposed per-(b,h) and
transposes cur_attn back while unsharding.  The softmax denominator comes
for free from a ones-column appended to V in the PV matmul, so no explicit
row-max/row-sum reduction passes are needed (logits here are O(1), exp is
safe without max subtraction).

Sharding: 8 cores = (4 batches) x (2 head-groups of 6 heads).  No
device-to-device communication; the final proj partial-sum pair-reduction
(+ proj bias) happens on the host during unsharding.
"""

import sys

sys.path.insert(0, "/opt/trn_rl_repo")

from contextlib import ExitStack

import ml_dtypes
import numpy as np

import concourse.bass as bass
import concourse.tile as tile
from concourse import bacc, mybir
from concourse.bass_utils import run_bass_kernel_spmd

B, N, C, H, HD = 4, 1024, 768, 12, 64
HPC = 6  # heads per core
NCORES = 8
P = 128
CT = C // P  # 6 contraction tiles
MT = N // P  # 8 m tiles
SCALE = HD ** -0.5

DT = mybir.dt
BF16 = DT.bfloat16
F32 = DT.float32
AF = mybir.ActivationFunctionType
ALU = mybir.AluOpType
BF = ml_dtypes.bfloat16


def _body(ctx: ExitStack, tc, io):
    nc = tc.nc

    persist = ctx.enter_context(tc.tile_pool(name="persist", bufs=1))
    ps = ctx.enter_context(tc.tile_pool(name="ps", bufs=2, space="PSUM"))
    ps_o = ctx.enter_context(tc.tile_pool(name="ps_o", bufs=2, space="PSUM"))
    qk_pool = ctx.enter_context(tc.tile_pool(name="qk", bufs=2))
    pt_pool = ctx.enter_context(tc.tile_pool(name="pt", bufs=12))
    prev_pool = ctx.enter_context(tc.tile_pool(name="prev", bufs=4))
    cur_pool = ctx.enter_context(tc.tile_pool(name="cur", bufs=4))
    bias_pool = ctx.enter_context(tc.tile_pool(name="bias", bufs=2))
    wb_pool = ctx.enter_context(tc.tile_pool(name="wb", bufs=6))
    tmp_pool = ctx.enter_context(tc.tile_pool(name="tmp", bufs=4))
    rows_pool = ctx.enter_context(tc.tile_pool(name="rows", bufs=2))
    rb_pool = ctx.enter_context(tc.tile_pool(name="rb", bufs=2))

    # ---- persistent inputs ----
    xt = []
    for c in range(CT):
        t = persist.tile([P, N], BF16, tag=f"xt{c}", name=f"xt{c}")
        nc.sync.dma_start(t[:], io["xT"][c * P:(c + 1) * P, :])
        xt.append(t)
    wqk_sb = []
    for c in range(CT):
        t = persist.tile([P, HPC * 128], BF16, tag=f"wqk{c}", name=f"wqk{c}")
        nc.sync.dma_start(t[:], io["wqk"][c * P:(c + 1) * P, :])
        wqk_sb.append(t)
    wv_sb = []
    for c in range(CT):
        t = persist.tile([P, HPC * HD], BF16, tag=f"wv{c}", name=f"wv{c}")
        nc.sync.dma_start(t[:], io["wv"][c * P:(c + 1) * P, :])
        wv_sb.append(t)
    pw_sb = []
    for g in range(3):
        t = persist.tile([P, C], BF16, tag=f"pw{g}", name=f"pw{g}")
        nc.sync.dma_start(t[:], io["pwT"][g * P:(g + 1) * P, :])
        pw_sb.append(t)
    w1Ta = persist.tile([64, 32], BF16, tag="w1Ta", name="w1Ta")
    nc.sync.dma_start(w1Ta[:], io["w1Ta"][:, :])
    w1Tb = persist.tile([64, 32], BF16, tag="w1Tb", name="w1Tb")
    nc.sync.dma_start(w1Tb[:], io["w1Tb"][:, :])
    b1_sb = persist.tile([32, 1], F32, tag="b1", name="b1_sb")
    nc.sync.dma_start(b1_sb[:], io["b1"][:, :])
    w2T_sb = persist.tile([32, 3], BF16, tag="w2T", name="w2T_sb")
    nc.sync.dma_start(w2T_sb[:], io["w2T"][:, :])
    b2_sb = persist.tile([3, 1], F32, tag="b2", name="b2_sb")
    nc.sync.dma_start(b2_sb[:], io["b2"][:, :])

    # ---- V in natural [m, d] layout, ones-augmented per head (col 64 of
    # each 65-wide head block is 1.0 so the PV matmul also yields rowsums) ----
    va = []
    for mt in range(MT):
        t = persist.tile([P, HPC * 65], BF16, tag=f"va{mt}", name=f"va{mt}")
        vps = ps.tile([P, N], F32, tag="s", name="vps")
        for c in range(CT):
            nc.tensor.matmul(
                vps[:, 0:HPC * HD],
                lhsT=xt[c][:, mt * P:(mt + 1) * P],
                rhs=wv_sb[c][:],
                start=(c == 0),
                stop=(c == CT - 1),
            )
        for h in range(HPC):
            nc.scalar.copy(t[:, h * 65:h * 65 + 64], vps[:, h * 64:(h + 1) * 64])
            nc.vector.memset(t[:, h * 65 + 64:h * 65 + 65], 1.0)
        va.append(t)

    # ---- colsum of V per head via xsum:
    # colsum_v[h, d] = sum_c W_v[h*64+d, c] * (sum_m x[m, c]) ----
    xsum = persist.tile([P, CT], F32, tag="xsum", name="xsum")
    xsumb = persist.tile([P, CT], BF16, tag="xsumb", name="xsumb")
    for c in range(CT):
        nc.vector.tensor_reduce(
            xsum[:, c:c + 1], xt[c][:], axis=mybir.AxisListType.X, op=ALU.add
        )
    nc.vector.tensor_copy(xsumb[:], xsum[:])
    cv = persist.tile([64, HPC], F32, tag="cv", name="cv")  # col h: colsum_v
    for h in range(HPC):
        cps = ps.tile([P, N], F32, tag="s", name="cps")
        for c in range(CT):
            nc.tensor.matmul(
                cps[0:64, 0:1],
                lhsT=wv_sb[c][:, h * HD:(h + 1) * HD],
                rhs=xsumb[:, c:c + 1],
                start=(c == 0),
                stop=(c == CT - 1),
            )
        nc.scalar.copy(cv[:, h:h + 1], cps[0:64, 0:1])

    O_all = [
        persist.tile([P, N], BF16, tag=f"oall{g}", name=f"oall{g}") for g in range(3)
    ]

    # ---- head loop ----
    for h in range(HPC):
        # qT / kT  [64, N]
        psq = ps.tile([P, N], F32, tag="s", name="psq")
        for c in range(CT):
            for nh in range(2):
                nc.tensor.matmul(
                    psq[0:64, nh * 512:(nh + 1) * 512],
                    lhsT=wqk_sb[c][:, h * 128:h * 128 + 64],
                    rhs=xt[c][:, nh * 512:(nh + 1) * 512],
                    start=(c == 0),
                    stop=(c == CT - 1),
                )
        qT = qk_pool.tile([64, N], BF16, tag="qT", name="qT")
        nc.scalar.copy(qT[:], psq[0:64, :])
        psk = ps.tile([P, N], F32, tag="s", name="psk")
        for c in range(CT):
            for nh in range(2):
                nc.tensor.matmul(
                    psk[0:64, nh * 512:(nh + 1) * 512],
                    lhsT=wqk_sb[c][:, h * 128 + 64:(h + 1) * 128],
                    rhs=xt[c][:, nh * 512:(nh + 1) * 512],
                    start=(c == 0),
                    stop=(c == CT - 1),
                )
        kT = qk_pool.tile([64, N], BF16, tag="kT", name="kT")
        nc.scalar.copy(kT[:], psk[0:64, :])

        # bias MLP: h1^T = relu(W1 @ qkcat^T + b1); biasT = abs(W2 @ h1^T + b2)
        ps1 = ps.tile([P, N], F32, tag="s", name="ps1")
        for nh in range(2):
            nc.tensor.matmul(
                ps1[0:32, nh * 512:(nh + 1) * 512],
                lhsT=w1Ta[:],
                rhs=qT[:, nh * 512:(nh + 1) * 512],
                start=True,
                stop=False,
            )
            nc.tensor.matmul(
                ps1[0:32, nh * 512:(nh + 1) * 512],
                lhsT=w1Tb[:],
                rhs=kT[:, nh * 512:(nh + 1) * 512],
                start=False,
                stop=True,
            )
        h1 = bias_pool.tile([32, N], BF16, tag="h1", name="h1")
        nc.scalar.activation(h1[:], ps1[0:32, :], AF.Relu, bias=b1_sb[:])
        ps2 = ps.tile([P, N], F32, tag="s", name="ps2")
        for nh in range(2):
            nc.tensor.matmul(
                ps2[0:3, nh * 512:(nh + 1) * 512],
                lhsT=w2T_sb[:],
                rhs=h1[:, nh * 512:(nh + 1) * 512],
                start=True,
                stop=True,
            )
        biasT = bias_pool.tile([3, N], BF16, tag="biasT", name="biasT")
        nc.scalar.activation(biasT[:], ps2[0:3, :], AF.Abs, bias=b2_sb[:])

        # rows 1/2 of biasT live on partitions 1/2; engines are lane-aligned,
        # so shift them to partition 0 via SBUF->SBUF DMA before broadcasting.
        b1row = rows_pool.tile([1, N], BF16, tag="b1row", name="b1row")
        nc.sync.dma_start(b1row[:], biasT[1:2, :])
        b2row = rows_pool.tile([1, N], BF16, tag="b2row", name="b2row")
        nc.sync.dma_start(b2row[:], biasT[2:3, :])
        w2b = wb_pool.tile([64, N], BF16, tag="wb", name="w2b")
        nc.gpsimd.partition_broadcast(w2b[:], b1row[:])
        nc.vector.tensor_scalar_mul(b2row[:], b2row[:], 1.0 / N)
        w3b = wb_pool.tile([64, N], BF16, tag="wb", name="w3b")
        nc.gpsimd.partition_broadcast(w3b[:], b2row[:])

        # scores + exp + PV over m tiles
        o1 = ps_o.tile([65, N], F32, tag="o", name="o1")
        o2 = ps_o.tile([65, N], F32, tag="o", name="o2")
        pts = []
        for mt in range(MT):
            pss = ps.tile([P, N], F32, tag="s", name="pss")
            for nh in range(2):
                nc.tensor.matmul(
                    pss[:, nh * 512:(nh + 1) * 512],
                    lhsT=kT[:, mt * P:(mt + 1) * P],
                    rhs=qT[:, nh * 512:(nh + 1) * 512],
                    start=True,
                    stop=True,
                )
            pt = pt_pool.tile([P, N], BF16, tag="pt", name="pt")
            nc.scalar.activation(pt[:], pss[:], AF.Exp, scale=SCALE)
            pts.append(pt)
            for nh in range(2):
                nc.tensor.matmul(
                    o1[:, nh * 512:(nh + 1) * 512],
                    lhsT=va[mt][:, h * 65:h * 65 + 65],
                    rhs=pt[:, nh * 512:(nh + 1) * 512],
                    start=(mt == 0),
                    stop=(mt == MT - 1),
                )
            pv = prev_pool.tile([P, N], BF16, tag="prev", name="pv")
            nc.sync.dma_start(pv[:], io["prevT"][h * N + mt * P:h * N + (mt + 1) * P, :])
            for nh in range(2):
                nc.tensor.matmul(
                    o2[0:64, nh * 512:(nh + 1) * 512],
                    lhsT=va[mt][:, h * 65:h * 65 + 64],
                    rhs=pv[:, nh * 512:(nh + 1) * 512],
                    start=(mt == 0),
                    stop=(mt == MT - 1),
                )

        # s = o1 row 64: ACT copy to SBUF lane 64, DMA-shift to partition 0
        s64 = rows_pool.tile([65, N], F32, tag="s64", name="s64")
        nc.scalar.copy(s64[64:65, :], o1[64:65, :])
        srow = rows_pool.tile([1, N], F32, tag="srow", name="srow")
        nc.sync.dma_start(srow[:], s64[64:65, :])
        recip = rows_pool.tile([1, N], F32, tag="recip", name="recip")
        nc.vector.reciprocal(recip[:], srow[:])
        w1row = rows_pool.tile([1, N], BF16, tag="w1row", name="w1row")
        nc.vector.tensor_scalar_add(w1row[:], biasT[0:1, :], 1.0)
        nc.vector.tensor_mul(w1row[:], w1row[:], recip[:])
        w1b = wb_pool.tile([64, N], BF16, tag="wb", name="w1b")
        nc.gpsimd.partition_broadcast(w1b[:], w1row[:])
        rb = rb_pool.tile([P, N], F32, tag="rb", name="rb")
        nc.gpsimd.partition_broadcast(rb[:], recip[:])

        # O assembly: Os = w1*O1 + w2*O2 - cv*w3   [64, N], then place into
        # O_all[g] partitions po..po+64 (partition shift => SBUF->SBUF DMA)
        g, po = h // 2, (h % 2) * 64
        t1 = tmp_pool.tile([64, N], F32, tag="t", name="t1")
        nc.vector.tensor_mul(t1[:], o1[0:64, :], w1b[:])
        t2 = tmp_pool.tile([64, N], F32, tag="t", name="t2")
        nc.vector.tensor_mul(t2[:], o2[0:64, :], w2b[:])
        t3 = tmp_pool.tile([64, N], F32, tag="t", name="t3")
        nc.vector.tensor_add(t3[:], t1[:], t2[:])
        m3 = tmp_pool.tile([64, N], F32, tag="t", name="m3")
        nc.vector.tensor_scalar(m3[:], w3b[:], cv[:, h:h + 1], None, ALU.mult)
        osl = tmp_pool.tile([64, N], BF16, tag="osl", name="osl")
        nc.vector.tensor_sub(osl[:], t3[:], m3[:])
        nc.sync.dma_start(O_all[g][po:po + 64, :], osl[:])

        # normalize P^T -> cur_attn^T and store
        for mt in range(MT):
            curt = cur_pool.tile([P, N], BF16, tag="cur", name="curt")
            nc.vector.tensor_mul(curt[:], pts[mt][:], rb[:])
            nc.sync.dma_start(
                io["curT"][h * N + mt * P:h * N + (mt + 1) * P, :], curt[:]
            )

    # ---- proj: outp^T[co, n] = sum_dstack pwT[dstack, co] * O_all[dstack, n] ----
    for co in range(CT):
        for nh in range(2):
            pp = ps.tile([P, N], F32, tag="s", name="pp")
            for g in range(3):
                nc.tensor.matmul(
                    pp[:, 0:512],
                    lhsT=pw_sb[g][:, co * P:(co + 1) * P],
                    rhs=O_all[g][:, nh * 512:(nh + 1) * 512],
                    start=(g == 0),
                    stop=(g == 2),
                )
            ot = cur_pool.tile([P, 512], F32, tag="ot", name="ot")
            nc.scalar.copy(ot[:], pp[:, 0:512])
            nc.sync.dma_start(
                io["outp"][co * P:(co + 1) * P, nh * 512:(nh + 1) * 512], ot[:]
            )


_GRAPH = None


def _build():
    global _GRAPH
    if _GRAPH is not None:
        return _GRAPH
    nc = bacc.Bacc("TRN2", target_bir_lowering=False, debug=False, num_devices=NCORES)
    io = {
        "xT": nc.dram_tensor("xT", [C, N], BF16, kind="ExternalInput").ap(),
        "wqk": nc.dram_tensor("wqk", [C, HPC * 128], BF16, kind="ExternalInput").ap(),
        "wv": nc.dram_tensor("wv", [C, HPC * HD], BF16, kind="ExternalInput").ap(),
        "pwT": nc.dram_tensor("pwT", [HPC * HD, C], BF16, kind="ExternalInput").ap(),
        "prevT": nc.dram_tensor("prevT", [HPC * N, N], BF16, kind="ExternalInput").ap(),
        "w1Ta": nc.dram_tensor("w1Ta", [64, 32], BF16, kind="ExternalInput").ap(),
        "w1Tb": nc.dram_tensor("w1Tb", [64, 32], BF16, kind="ExternalInput").ap(),
        "b1": nc.dram_tensor("b1", [32, 1], F32, kind="ExternalInput").ap(),
        "w2T": nc.dram_tensor("w2T", [32, 3], BF16, kind="ExternalInput").ap(),
        "b2": nc.dram_tensor("b2", [3, 1], F32, kind="ExternalInput").ap(),
        "curT": nc.dram_tensor("curT", [HPC * N, N], BF16, kind="ExternalOutput").ap(),
        "outp": nc.dram_tensor("outp", [C, N], F32, kind="ExternalOutput").ap(),
    }
    with tile.TileContext(nc) as tc:
        with ExitStack() as ctx:
            _body(ctx, tc, io)
    nc.compile()
    _GRAPH = nc
    return nc


def make_in_maps(x, prev_attn, qkv_w, proj_w, bp_w1, bp_b1, bp_w2, bp_b2):
    in_maps = []
    qr = np.asarray(qkv_w).reshape(3, H, HD, C)
    w1 = np.asarray(bp_w1)  # [32, 128]
    for core in range(NCORES):
        b, g = core // 2, core % 2
        hs = slice(g * HPC, (g + 1) * HPC)
        wqk = np.concatenate([qr[0, hs], qr[1, hs]], axis=1)  # [6, 128, C]
        in_maps.append(
            {
                "xT": np.ascontiguousarray(np.asarray(x)[b].T).astype(BF),
                "wqk": np.ascontiguousarray(wqk.reshape(HPC * 128, C).T).astype(BF),
                "wv": np.ascontiguousarray(qr[2, hs].reshape(HPC * HD, C).T).astype(BF),
                "pwT": np.ascontiguousarray(
                    np.asarray(proj_w)[:, g * HPC * HD:(g + 1) * HPC * HD].T
                ).astype(BF),
                "prevT": np.ascontiguousarray(
                    np.asarray(prev_attn)[b, hs].transpose(0, 2, 1)
                ).reshape(HPC * N, N).astype(BF),
                "w1Ta": np.ascontiguousarray(w1[:, 0:64].T).astype(BF),
                "w1Tb": np.ascontiguousarray(w1[:, 64:128].T).astype(BF),
                "b1": np.asarray(bp_b1).reshape(32, 1).astype(np.float32),
                "w2T": np.ascontiguousarray(np.asarray(bp_w2).T).astype(BF),
                "b2": np.asarray(bp_b2).reshape(3, 1).astype(np.float32),
            }
        )
    return in_maps


def run_on_device(in_maps, **kw):
    nc = _build()
    return run_bass_kernel_spmd(nc, in_maps, core_ids=list(range(NCORES)), **kw)


def assemble(results, proj_b):
    cur = np.empty((B, H, N, N), np.float32)
    out = np.empty((B, N, C), np.float32)
    for core in range(NCORES):
        b, g = core // 2, core % 2
        ct = results[core]["curT"].astype(np.float32).reshape(HPC, N, N)
        cur[b, g * HPC:(g + 1) * HPC] = ct.transpose(0, 2, 1)
    pb = np.asarray(proj_b).astype(np.float32)
    for b in range(B):
        o = results[2 * b]["outp"].astype(np.float32) + results[2 * b + 1][
            "outp"
        ].astype(np.float32)
        out[b] = o.T + pb[None, :]
    return out, cur


def kernel(x, prev_attn, qkv_w, proj_w, proj_b, bp_w1, bp_b1, bp_w2, bp_b2):
    in_maps = make_in_maps(x, prev_attn, qkv_w, proj_w, bp_w1, bp_b1, bp_w2, bp_b2)
    res = run_on_device(in_maps)
    return assemble(res.results, proj_b)


# revision 7
# speedup vs baseline: 1.1872x; 1.1872x over previous
"""Trainium2 Bass kernel for the blended-attention module.

Math (per batch b, head h):
  q,k,v = per-head projections of x           [N, 64]
  S = q @ k.T * scale                         [N, N]
  cur = softmax(S, axis=-1)
  bias = abs(relu(concat(q,k) @ w1.T + b1) @ w2.T + b2)   [N, 3]  (per-row)
  attn = cur*(1+bias0) + prev*bias1 - bias2/N
  out_head = attn @ v ;  out = concat_heads @ proj_w.T + proj_b
Returns (out, cur).

Key restructuring: bias0/1/2 are per-row scalars, so
  out_head = diag((1+b0)/s) @ (P@v) + diag(b1) @ (prev@v) - diag(b2/N) @ (1.T@v)
with P = exp(S*scale) unnormalized, s = rowsum(P).  No blended-attn
materialization; prev_attn is consumed only by a matmul.

Everything on-device is computed in the transposed orientation (S^T [m, n],
P^T, O^T) so the PE contraction axes line up without any on-device
transposes; the host feeds prev_attn pre-transposed per-(b,h) and
transposes cur_attn back while unsharding.  The softmax denominator comes
for free from a ones-column appended to V in the PV matmul, so no explicit
row-max/row-sum reduction passes are needed (logits here are O(1), exp is
safe without max subtraction).

Sharding: 8 cores = (4 batches) x (2 head-groups of 6 heads).  No
device-to-device communication; the final proj partial-sum pair-reduction
(+ proj bias) happens on the host during unsharding.
"""

import sys

sys.path.insert(0, "/opt/trn_rl_repo")

from contextlib import ExitStack

import ml_dtypes
import numpy as np

import concourse.bass as bass
import concourse.tile as tile
from concourse import bacc, mybir
from concourse.bass_utils import run_bass_kernel_spmd

B, N, C, H, HD = 4, 1024, 768, 12, 64
HPC = 6  # heads per core
NCORES = 8
P = 128
CT = C // P  # 6 contraction tiles
MT = N // P  # 8 m tiles
SCALE = HD ** -0.5

DT = mybir.dt
BF16 = DT.bfloat16
F32 = DT.float32
AF = mybir.ActivationFunctionType
ALU = mybir.AluOpType
BF = ml_dtypes.bfloat16


def _body(ctx: ExitStack, tc, io):
    nc = tc.nc

    persist = ctx.enter_context(tc.tile_pool(name="persist", bufs=1))
    ps = ctx.enter_context(tc.tile_pool(name="ps", bufs=2, space="PSUM"))
    ps_o = ctx.enter_context(tc.tile_pool(name="ps_o", bufs=2, space="PSUM"))
    qk_pool = ctx.enter_context(tc.tile_pool(name="qk", bufs=2))
    pt_pool = ctx.enter_context(tc.tile_pool(name="pt", bufs=12))
    prev_pool = ctx.enter_context(tc.tile_pool(name="prev", bufs=4))
    cur_pool = ctx.enter_context(tc.tile_pool(name="cur", bufs=4))
    bias_pool = ctx.enter_context(tc.tile_pool(name="bias", bufs=2))
    wb_pool = ctx.enter_context(tc.tile_pool(name="wb", bufs=6))
    tmp_pool = ctx.enter_context(tc.tile_pool(name="tmp", bufs=4))
    rows_pool = ctx.enter_context(tc.tile_pool(name="rows", bufs=2))
    rb_pool = ctx.enter_context(tc.tile_pool(name="rb", bufs=2))

    # ---- persistent inputs ----
    xt = []
    for c in range(CT):
        t = persist.tile([P, N], BF16, tag=f"xt{c}", name=f"xt{c}")
        nc.sync.dma_start(t[:], io["xT"][c * P:(c + 1) * P, :])
        xt.append(t)
    wqk_sb = []
    for c in range(CT):
        t = persist.tile([P, HPC * 128], BF16, tag=f"wqk{c}", name=f"wqk{c}")
        nc.sync.dma_start(t[:], io["wqk"][c * P:(c + 1) * P, :])
        wqk_sb.append(t)
    wv_sb = []
    for c in range(CT):
        t = persist.tile([P, HPC * HD], BF16, tag=f"wv{c}", name=f"wv{c}")
        nc.sync.dma_start(t[:], io["wv"][c * P:(c + 1) * P, :])
        wv_sb.append(t)
    pw_sb = []
    for g in range(3):
        t = persist.tile([P, C], BF16, tag=f"pw{g}", name=f"pw{g}")
        nc.sync.dma_start(t[:], io["pwT"][g * P:(g + 1) * P, :])
        pw_sb.append(t)
    w1T_sb = persist.tile([P, 32], BF16, tag="w1T", name="w1T_sb")
    nc.sync.dma_start(w1T_sb[:], io["w1T"][:, :])
    b1_sb = persist.tile([32, 1], F32, tag="b1", name="b1_sb")
    nc.sync.dma_start(b1_sb[:], io["b1"][:, :])
    w2T_sb = persist.tile([32, 3], BF16, tag="w2T", name="w2T_sb")
    nc.sync.dma_start(w2T_sb[:], io["w2T"][:, :])
    b2_sb = persist.tile([3, 1], F32, tag="b2", name="b2_sb")
    nc.sync.dma_start(b2_sb[:], io["b2"][:, :])

    # ---- V in natural [m, d] layout, ones-augmented per head (col 64 of
    # each 65-wide head block is 1.0 so the PV matmul also yields rowsums) ----
    va = []
    for mt in range(MT):
        t = persist.tile([P, HPC * 65], BF16, tag=f"va{mt}", name=f"va{mt}")
        vps = ps.tile([P, N], F32, tag="s", name="vps")
        for c in range(CT):
            nc.tensor.matmul(
                vps[:, 0:HPC * HD],
                lhsT=xt[c][:, mt * P:(mt + 1) * P],
                rhs=wv_sb[c][:],
                start=(c == 0),
                stop=(c == CT - 1),
            )
        for h in range(HPC):
            nc.scalar.copy(t[:, h * 65:h * 65 + 64], vps[:, h * 64:(h + 1) * 64])
            nc.vector.memset(t[:, h * 65 + 64:h * 65 + 65], 1.0)
        va.append(t)

    # ---- colsum of V per head via xsum:
    # colsum_v[h, d] = sum_c W_v[h*64+d, c] * (sum_m x[m, c]) ----
    xsum = persist.tile([P, CT], F32, tag="xsum", name="xsum")
    xsumb = persist.tile([P, CT], BF16, tag="xsumb", name="xsumb")
    for c in range(CT):
        nc.vector.tensor_reduce(
            xsum[:, c:c + 1], xt[c][:], axis=mybir.AxisListType.X, op=ALU.add
        )
    nc.vector.tensor_copy(xsumb[:], xsum[:])
    cv = persist.tile([64, HPC], F32, tag="cv", name="cv")  # col h: colsum_v
    for h in range(HPC):
        cps = ps.tile([P, N], F32, tag="s", name="cps")
        for c in range(CT):
            nc.tensor.matmul(
                cps[0:64, 0:1],
                lhsT=wv_sb[c][:, h * HD:(h + 1) * HD],
                rhs=xsumb[:, c:c + 1],
                start=(c == 0),
                stop=(c == CT - 1),
            )
        nc.scalar.copy(cv[:, h:h + 1], cps[0:64, 0:1])
    # fold the uniform-term 1/N into cv so w3b is just the bias2 row
    nc.vector.tensor_scalar_mul(cv[:], cv[:], 1.0 / N)

    O_all = [
        persist.tile([P, N], BF16, tag=f"oall{g}", name=f"oall{g}") for g in range(3)
    ]

    # ---- head loop ----
    for h in range(HPC):
        # packed q/k projection: psq rows 0:64 = Q^T, 64:128 = K^T
        psq = ps.tile([P, N], F32, tag="s", name="psq")
        for c in range(CT):
            for nh in range(2):
                nc.tensor.matmul(
                    psq[:, nh * 512:(nh + 1) * 512],
                    lhsT=wqk_sb[c][:, h * 128:(h + 1) * 128],
                    rhs=xt[c][:, nh * 512:(nh + 1) * 512],
                    start=(c == 0),
                    stop=(c == CT - 1),
                )
        qk = qk_pool.tile([P, N], BF16, tag="qk", name="qk")
        nc.scalar.copy(qk[:], psq[:])
        # K^T must sit at partition base 0 to be the scores lhsT -> DMA shift
        kT = qk_pool.tile([64, N], BF16, tag="kT", name="kT")
        nc.sync.dma_start(kT[:], qk[64:128, :])

        # bias MLP: h1^T = relu(W1 @ qkcat^T + b1); biasT = abs(W2 @ h1^T + b2)
        ps1 = ps.tile([P, N], F32, tag="s", name="ps1")
        for nh in range(2):
            nc.tensor.matmul(
                ps1[0:32, nh * 512:(nh + 1) * 512],
                lhsT=w1T_sb[:],
                rhs=qk[:, nh * 512:(nh + 1) * 512],
                start=True,
                stop=True,
            )
        h1 = bias_pool.tile([32, N], BF16, tag="h1", name="h1")
        nc.scalar.activation(h1[:], ps1[0:32, :], AF.Relu, bias=b1_sb[:])
        ps2 = ps.tile([P, N], F32, tag="s", name="ps2")
        for nh in range(2):
            nc.tensor.matmul(
                ps2[0:3, nh * 512:(nh + 1) * 512],
                lhsT=w2T_sb[:],
                rhs=h1[:, nh * 512:(nh + 1) * 512],
                start=True,
                stop=True,
            )
        biasT = bias_pool.tile([3, N], BF16, tag="biasT", name="biasT")
        nc.scalar.activation(biasT[:], ps2[0:3, :], AF.Abs, bias=b2_sb[:])

        # rows 1/2 of biasT live on partitions 1/2; engines are lane-aligned,
        # so shift them to partition 0 via SBUF->SBUF DMA before broadcasting.
        b1row = rows_pool.tile([1, N], BF16, tag="b1row", name="b1row")
        nc.sync.dma_start(b1row[:], biasT[1:2, :])
        b2row = rows_pool.tile([1, N], BF16, tag="b2row", name="b2row")
        nc.sync.dma_start(b2row[:], biasT[2:3, :])
        w2b = wb_pool.tile([64, N], BF16, tag="wb", name="w2b")
        nc.gpsimd.partition_broadcast(w2b[:], b1row[:])
        w3b = wb_pool.tile([64, N], BF16, tag="wb", name="w3b")
        nc.gpsimd.partition_broadcast(w3b[:], b2row[:])

        # scores + exp + PV over m tiles
        o1 = ps_o.tile([65, N], F32, tag="o", name="o1")
        o2 = ps_o.tile([65, N], F32, tag="o", name="o2")
        pts = []
        for mt in range(MT):
            pss = ps.tile([P, N], F32, tag="s", name="pss")
            for nh in range(2):
                nc.tensor.matmul(
                    pss[:, nh * 512:(nh + 1) * 512],
                    lhsT=kT[:, mt * P:(mt + 1) * P],
                    rhs=qk[0:64, nh * 512:(nh + 1) * 512],
                    start=True,
                    stop=True,
                )
            pt = pt_pool.tile([P, N], BF16, tag="pt", name="pt")
            nc.scalar.activation(pt[:], pss[:], AF.Exp, scale=SCALE)
            pts.append(pt)
            for nh in range(2):
                nc.tensor.matmul(
                    o1[:, nh * 512:(nh + 1) * 512],
                    lhsT=va[mt][:, h * 65:h * 65 + 65],
                    rhs=pt[:, nh * 512:(nh + 1) * 512],
                    start=(mt == 0),
                    stop=(mt == MT - 1),
                )
            pv = prev_pool.tile([P, N], BF16, tag="prev", name="pv")
            nc.sync.dma_start(pv[:], io["prevT"][h * N + mt * P:h * N + (mt + 1) * P, :])
            for nh in range(2):
                nc.tensor.matmul(
                    o2[0:64, nh * 512:(nh + 1) * 512],
                    lhsT=va[mt][:, h * 65:h * 65 + 64],
                    rhs=pv[:, nh * 512:(nh + 1) * 512],
                    start=(mt == 0),
                    stop=(mt == MT - 1),
                )

        # drain o1/o2 out of PSUM immediately (frees the slots for the next
        # head) and keep s in f32 for the reciprocal
        s64 = rows_pool.tile([65, N], F32, tag="s64", name="s64")
        nc.scalar.copy(s64[64:65, :], o1[64:65, :])
        o1sb = tmp_pool.tile([64, N], BF16, tag="o1sb", name="o1sb")
        nc.scalar.copy(o1sb[:], o1[0:64, :])
        o2sb = tmp_pool.tile([64, N], BF16, tag="o2sb", name="o2sb")
        nc.scalar.copy(o2sb[:], o2[0:64, :])

        # reciprocal of s: single-lane [1,N] recip is ~6.5us on DVE, so DMA-
        # reshape to [128,8], recip there, reshape back
        srt = rows_pool.tile([P, 8], F32, tag="srt", name="srt")
        nc.sync.dma_start(srt[:], s64[64:65, :])
        rrt = rows_pool.tile([P, 8], F32, tag="rrt", name="rrt")
        nc.vector.reciprocal(rrt[:], srt[:])
        rrtb = rows_pool.tile([P, 8], BF16, tag="rrtb", name="rrtb")
        nc.vector.tensor_copy(rrtb[:], rrt[:])
        b0rt = rows_pool.tile([P, 8], BF16, tag="b0rt", name="b0rt")
        nc.sync.dma_start(b0rt[:], biasT[0:1, :])
        w1rt = rows_pool.tile([P, 8], BF16, tag="w1rt", name="w1rt")
        nc.vector.tensor_scalar_add(w1rt[:], b0rt[:], 1.0)
        nc.vector.tensor_mul(w1rt[:], w1rt[:], rrtb[:])
        w1row = rows_pool.tile([1, N], BF16, tag="w1row", name="w1row")
        nc.sync.dma_start(w1row[:], w1rt[:])
        rbrow = rows_pool.tile([1, N], BF16, tag="rbrow", name="rbrow")
        nc.sync.dma_start(rbrow[:], rrtb[:])
        w1b = wb_pool.tile([64, N], BF16, tag="wb", name="w1b")
        nc.gpsimd.partition_broadcast(w1b[:], w1row[:])
        rb = rb_pool.tile([P, N], BF16, tag="rb", name="rb")
        nc.gpsimd.partition_broadcast(rb[:], rbrow[:])

        # O assembly: Os = w1*O1 + w2*O2 - cv*w3   [64, N], then place into
        # O_all[g] partitions po..po+64 (partition shift => SBUF->SBUF DMA)
        g, po = h // 2, (h % 2) * 64
        t1 = tmp_pool.tile([64, N], BF16, tag="t", name="t1")
        nc.vector.tensor_mul(t1[:], o1sb[:], w1b[:])
        t2 = tmp_pool.tile([64, N], BF16, tag="t", name="t2")
        nc.vector.tensor_mul(t2[:], o2sb[:], w2b[:])
        t3 = tmp_pool.tile([64, N], BF16, tag="t", name="t3")
        nc.vector.tensor_add(t3[:], t1[:], t2[:])
        m3 = tmp_pool.tile([64, N], BF16, tag="t", name="m3")
        nc.vector.tensor_scalar(m3[:], w3b[:], cv[:, h:h + 1], None, ALU.mult)
        osl = tmp_pool.tile([64, N], BF16, tag="osl", name="osl")
        nc.vector.tensor_sub(osl[:], t3[:], m3[:])
        nc.sync.dma_start(O_all[g][po:po + 64, :], osl[:])

        # normalize P^T -> cur_attn^T and store
        for mt in range(MT):
            curt = cur_pool.tile([P, N], BF16, tag="cur", name="curt")
            nc.vector.tensor_mul(curt[:], pts[mt][:], rb[:])
            nc.sync.dma_start(
                io["curT"][h * N + mt * P:h * N + (mt + 1) * P, :], curt[:]
            )

    # ---- proj: outp^T[co, n] = sum_dstack pwT[dstack, co] * O_all[dstack, n] ----
    for co in range(CT):
        for nh in range(2):
            pp = ps.tile([P, N], F32, tag="s", name="pp")
            for g in range(3):
                nc.tensor.matmul(
                    pp[:, 0:512],
                    lhsT=pw_sb[g][:, co * P:(co + 1) * P],
                    rhs=O_all[g][:, nh * 512:(nh + 1) * 512],
                    start=(g == 0),
                    stop=(g == 2),
                )
            ot = cur_pool.tile([P, 512], F32, tag="ot", name="ot")
            nc.scalar.copy(ot[:], pp[:, 0:512])
            nc.sync.dma_start(
                io["outp"][co * P:(co + 1) * P, nh * 512:(nh + 1) * 512], ot[:]
            )


_GRAPH = None


def _build():
    global _GRAPH
    if _GRAPH is not None:
        return _GRAPH
    nc = bacc.Bacc("TRN2", target_bir_lowering=False, debug=False, num_devices=NCORES)
    io = {
        "xT": nc.dram_tensor("xT", [C, N], BF16, kind="ExternalInput").ap(),
        "wqk": nc.dram_tensor("wqk", [C, HPC * 128], BF16, kind="ExternalInput").ap(),
        "wv": nc.dram_tensor("wv", [C, HPC * HD], BF16, kind="ExternalInput").ap(),
        "pwT": nc.dram_tensor("pwT", [HPC * HD, C], BF16, kind="ExternalInput").ap(),
        "prevT": nc.dram_tensor("prevT", [HPC * N, N], BF16, kind="ExternalInput").ap(),
        "w1T": nc.dram_tensor("w1T", [128, 32], BF16, kind="ExternalInput").ap(),
        "b1": nc.dram_tensor("b1", [32, 1], F32, kind="ExternalInput").ap(),
        "w2T": nc.dram_tensor("w2T", [32, 3], BF16, kind="ExternalInput").ap(),
        "b2": nc.dram_tensor("b2", [3, 1], F32, kind="ExternalInput").ap(),
        "curT": nc.dram_tensor("curT", [HPC * N, N], BF16, kind="ExternalOutput").ap(),
        "outp": nc.dram_tensor("outp", [C, N], F32, kind="ExternalOutput").ap(),
    }
    with tile.TileContext(nc) as tc:
        with ExitStack() as ctx:
            _body(ctx, tc, io)
    nc.compile()
    _GRAPH = nc
    return nc


def make_in_maps(x, prev_attn, qkv_w, proj_w, bp_w1, bp_b1, bp_w2, bp_b2):
    in_maps = []
    qr = np.asarray(qkv_w).reshape(3, H, HD, C)
    w1 = np.asarray(bp_w1)  # [32, 128]
    for core in range(NCORES):
        b, g = core // 2, core % 2
        hs = slice(g * HPC, (g + 1) * HPC)
        wqk = np.concatenate([qr[0, hs], qr[1, hs]], axis=1)  # [6, 128, C]
        in_maps.append(
            {
                "xT": np.ascontiguousarray(np.asarray(x)[b].T).astype(BF),
                "wqk": np.ascontiguousarray(wqk.reshape(HPC * 128, C).T).astype(BF),
                "wv": np.ascontiguousarray(qr[2, hs].reshape(HPC * HD, C).T).astype(BF),
                "pwT": np.ascontiguousarray(
                    np.asarray(proj_w)[:, g * HPC * HD:(g + 1) * HPC * HD].T
                ).astype(BF),
                "prevT": np.ascontiguousarray(
                    np.asarray(prev_attn)[b, hs].transpose(0, 2, 1)
                ).reshape(HPC * N, N).astype(BF),
                "w1T": np.ascontiguousarray(w1.T).astype(BF),
                "b1": np.asarray(bp_b1).reshape(32, 1).astype(np.float32),
                "w2T": np.ascontiguousarray(np.asarray(bp_w2).T).astype(BF),
                "b2": np.asarray(bp_b2).reshape(3, 1).astype(np.float32),
            }
        )
    return in_maps


def run_on_device(in_maps, **kw):
    nc = _build()
    return run_bass_kernel_spmd(nc, in_maps, core_ids=list(range(NCORES)), **kw)


def assemble(results, proj_b):
    cur = np.empty((B, H, N, N), np.float32)
    out = np.empty((B, N, C), np.float32)
    for core in range(NCORES):
        b, g = core // 2, core % 2
        ct = results[core]["curT"].astype(np.float32).reshape(HPC, N, N)
        cur[b, g * HPC:(g + 1) * HPC] = ct.transpose(0, 2, 1)
    pb = np.asarray(proj_b).astype(np.float32)
    for b in range(B):
        o = results[2 * b]["outp"].astype(np.float32) + results[2 * b + 1][
            "outp"
        ].astype(np.float32)
        out[b] = o.T + pb[None, :]
    return out, cur


def kernel(x, prev_attn, qkv_w, proj_w, proj_b, bp_w1, bp_b1, bp_w2, bp_b2):
    in_maps = make_in_maps(x, prev_attn, qkv_w, proj_w, bp_w1, bp_b1, bp_w2, bp_b2)
    res = run_on_device(in_maps)
    return assemble(res.results, proj_b)
